# revision 17
# baseline (speedup 1.0000x reference)
"""Trainium2 Bass kernel for the 4-layer adaLN causal transformer (v4).

v4 restructure: head-parallel attention with sequence-parallel dense.

Sharding: 8 cores = 2 batch groups x 4 cores. Core (b, r) owns contiguous
tokens [r*256, (r+1)*256) of batch b for the residual stream / adaLN / FFN,
and heads [4r, 4r+4) for attention. Per layer the collectives are ONE
AllGather of the modulated activations hT (f16, 0.5MB -> 2.1MB out) and ONE
ReduceScatter of the out-projection partials (f16, 0.5MB out) - replacing
v3's 4.7MB AllGather of k/v. Wqkv and Wout are head/row-sliced per core
(4x less weight DMA for those); W1/W2 replicated.

Attention per core: 4 heads x full causal triangle over 1024 tokens - zero
wasted key-block work (v3's uniform zigzag program wasted ~30%). The causal
diagonal is masked by accumulating a constant -30000 upper-triangle into the
e-PSUM via a second matmul (no DVE kill-mask pass). The softmax denominator
rides as a 65th ones-column on the v strips; a tiny extra matmul adds 1e-30
to the denominator row so fully-padded rows stay finite (no NaN).

adaLN affine folded into Wqkv/W1 on the host exactly as v3 (gamma scales
weight rows, beta -> q/k and gelu biases, v-beta -> bout). Residuals masked
every sub-block; activations f16, attention weights bf16, residual f32r.
"""

import os
import numpy as np

import concourse.bacc as bacc
import concourse.mybir as mybir
from concourse.tile import TileContext
from concourse.bass_utils import run_bass_kernel_spmd

F32 = mybir.dt.float32
F32R = mybir.dt.float32r
F16 = mybir.dt.float16
BF16 = mybir.dt.bfloat16
AF = mybir.ActivationFunctionType
ALU = mybir.AluOpType

D = 1024
T = 1024
L = 4
CH = 256            # own tokens per core
TG = 1024           # gathered tokens (whole batch)
KC = 8              # d_model chunks
ADALN_K = 0.1
EPS = 1e-5
SCALE = 0.125
RG = [[0, 1, 2, 3], [4, 5, 6, 7]]
REPS = int(os.environ.get("BK2_REPS", "1"))
SKIP_AG = bool(int(os.environ.get("BK2_SKIP_AG", "0")))
SKIP_FFN = bool(int(os.environ.get("BK2_SKIP_FFN", "0")))

_CACHED = {}


def _build_nc():
    nc = bacc.Bacc(target_bir_lowering=False, debug=False)

    xT_d = nc.dram_tensor("xT", [D, CH], F32, kind="ExternalInput")
    # mb order: q0, q1, k0, k1 (pairs of my 4 heads)
    wqk_d = nc.dram_tensor("wqk", [L, 4, 128, KC, 128], F16, kind="ExternalInput")
    wv_d = nc.dram_tensor("wv", [L, 128, KC, 256], F16, kind="ExternalInput")
    wout_d = nc.dram_tensor("wout", [L, 2, 128, 8, 128], F16, kind="ExternalInput")
    w1_d = nc.dram_tensor("w1", [L, 32, 128, KC, 128], F16, kind="ExternalInput")
    w2_d = nc.dram_tensor("w2", [L, 8, 128, 32, 128], F16, kind="ExternalInput")
    qkb_d = nc.dram_tensor("qkb", [L, 128, 4], F32, kind="ExternalInput")
    b1t_d = nc.dram_tensor("b1t", [L, 128, 32], F32, kind="ExternalInput")
    bsum_d = nc.dram_tensor("bsum", [L, 2, 128, 8], F32, kind="ExternalInput")
    mqp8_d = nc.dram_tensor("mqp8", [128, 8], F32, kind="ExternalInput")
    mrow_d = nc.dram_tensor("mrow", [1, CH], F32, kind="ExternalInput")
    ones_d = nc.dram_tensor("onescol", [128, 1], F32, kind="ExternalInput")
    kb_d = nc.dram_tensor("kbias", [128, 1], F32, kind="ExternalInput")
    diagm_d = nc.dram_tensor("diagm", [128, 512], F16, kind="ExternalInput")
    ident_d = nc.dram_tensor("identm", [128, 128], F16, kind="ExternalInput")
    tiny_d = nc.dram_tensor("tiny65", [128, 65], F32, kind="ExternalInput")
    out_d = nc.dram_tensor("out_xT", [D, CH], F32, kind="ExternalOutput")

    with TileContext(nc) as tc:
        with nc.allow_low_precision("fp16/bf16 intermediates by design"), \
             tc.tile_pool(name="pers", bufs=1) as pers, \
             tc.tile_pool(name="wqp", bufs=2) as wqp, \
             tc.tile_pool(name="wvp", bufs=2) as wvp, \
             tc.tile_pool(name="wop", bufs=2) as wop, \
             tc.tile_pool(name="wf1", bufs=2) as wf1, \
             tc.tile_pool(name="wf2", bufs=2) as wf2, \
             tc.tile_pool(name="cst", bufs=8) as cst, \
             tc.tile_pool(name="tp", bufs=3) as tp, \
             tc.tile_pool(name="tp4", bufs=3) as tp4, \
             tc.tile_pool(name="pmid", bufs=3, space="PSUM") as pmid, \
             tc.tile_pool(name="psS", bufs=3, space="PSUM") as psS, \
             tc.tile_pool(name="pso", bufs=2, space="PSUM") as pso, \
             tc.tile_pool(name="dr", bufs=2, space="DRAM") as dr:

            # ---- persistent tiles ----
            xT = pers.tile([128, KC * CH], F32R, tag="xT")
            hT = pers.tile([128, KC * CH], F16, tag="hT")
            hTg = pers.tile([128, KC * TG], F16, tag="hTg")   # chunk c at c*TG
            qT2 = pers.tile([128, 2 * TG], F16, tag="qT2")    # pair m at m*TG
            kT2 = pers.tile([128, 2 * TG], F16, tag="kT2")
            vstg = pers.tile([128, 8 * 260], BF16, tag="vstg")  # jb at jb*260, head h at +h*65
            oT = pers.tile([128, 2 * TG], F16, tag="oT")
            outP = pers.tile([128, 8 * TG], F16, tag="outP")
            ffT = pers.tile([128, 32 * CH], F16, tag="ffT")
            resT = pers.tile([128, KC * CH], F16, tag="resT")
            onesK = pers.tile([128, 1], F32R, tag="onesK")
            onesB = pers.tile([1, 128], F32R, tag="onesB")
            ones256 = pers.tile([128, 256], BF16, tag="ones256")
            tiny65 = pers.tile([128, 65], BF16, tag="tiny65")
            dmask2_t = pers.tile([128, 512], F16, tag="dmask2")
            ident_t = pers.tile([128, 128], F16, tag="ident")
            kb_t = pers.tile([128, 1], F32, tag="kb")
            mqp8_t = pers.tile([128, 8], F32, tag="mqp8")
            mrow_r = pers.tile([1, CH], F32R, tag="mrow")
            mbT = pers.tile([128, CH], F32, tag="mbT")

            nc.sync.dma_start(onesK[:, :], ones_d[:, :].bitcast(F32R))
            nc.sync.dma_start(onesB[:, :], ones_d[:, 0:1].bitcast(F32R).rearrange("p 1 -> 1 p"))
            nc.sync.dma_start(dmask2_t[:, :], diagm_d[:, :])
            nc.sync.dma_start(ident_t[:, :], ident_d[:, :])
            nc.sync.dma_start(kb_t[:, :], kb_d[:, :])
            nc.sync.dma_start(mqp8_t[:, :], mqp8_d[:, :])
            nc.sync.dma_start(mrow_r[:, :], mrow_d[:, :].bitcast(F32R))
            nc.vector.memset(ones256[:, :], 1.0)
            tinyf = tp.tile([128, 65], F32, tag="tinyf")
            nc.sync.dma_start(tinyf[:, :], tiny_d[:, :])
            nc.vector.tensor_copy(tiny65[:, :], tinyf[:, :])
            # v ones columns (x=64 of each 65-strip), masked by key validity
            for jb in range(8):
                ones_ap = (vstg[:, jb * 260:(jb + 1) * 260]
                           .rearrange("p (h x) -> p h x", x=65)[:, :, 64:65])
                nc.vector.memset(ones_ap, 1.0)
                nc.vector.tensor_scalar_mul(ones_ap, ones_ap, mqp8_t[:, jb:jb + 1])
            nc.sync.dma_start(
                xT[:, :].rearrange("q (c t) -> q c t", c=KC),
                xT_d[:, :].bitcast(F32R).rearrange("(c q) t -> q c t", q=128))
            # own-token mask broadcast [128, CH]
            pm = psS.tile([128, CH], F32, tag="ps")
            nc.tensor.matmul(pm[:, :], onesB[:, :], mrow_r[:, :], start=True, stop=True)
            nc.vector.tensor_copy(mbT[:, :], pm[:, :])

            consts = {}

            def load_layer_consts(layer):
                qkb = cst.tile([128, 4], F32, tag="qkb")
                nc.sync.dma_start(qkb[:, :], qkb_d[layer])
                b1t = cst.tile([128, 32], F32, tag="b1")
                nc.sync.dma_start(b1t[:, :], b1t_d[layer])
                bs0 = cst.tile([128, 8], F32, tag="bs0")
                nc.sync.dma_start(bs0[:, :], bsum_d[layer, 0])
                bs1 = cst.tile([128, 8], F32, tag="bs1")
                nc.sync.dma_start(bs1[:, :], bsum_d[layer, 1])
                consts.update({"qkb": qkb, "b1": b1t, 0: bs0, 1: bs1})

            def emit_adaln(layer, sb):
                """hT = mod(norm(xT)); affine folded into consuming weights."""
                ps_sum = psS.tile([1, CH], F32, tag="ps")
                ps_sq = psS.tile([1, CH], F32, tag="ps")
                for c in range(KC):
                    xs = xT[:, c * CH:(c + 1) * CH]
                    xsq = tp.tile([128, CH], F32R, tag="xsq")
                    nc.vector.tensor_tensor(xsq[:, :], xs, xs, ALU.mult)
                    nc.tensor.matmul(ps_sum[:, :], onesK[:, :], xs,
                                     start=(c == 0), stop=(c == KC - 1))
                    nc.tensor.matmul(ps_sq[:, :], onesK[:, :], xsq[:, :],
                                     start=(c == 0), stop=(c == KC - 1))
                murow = tp.tile([1, CH], F32R, tag="murow")
                nc.scalar.mul(murow[:, :], ps_sum[0:1, :], 1.0 / D)
                m2row = tp.tile([1, CH], F32, tag="m2row")
                nc.scalar.mul(m2row[:, :], ps_sq[0:1, :], 1.0 / D)
                musq = tp.tile([1, CH], F32, tag="musq")
                nc.vector.tensor_tensor(musq[:, :], murow[:, :], murow[:, :], ALU.mult)
                nc.vector.tensor_tensor(m2row[:, :], m2row[:, :], musq[:, :], ALU.subtract)
                nc.vector.tensor_scalar_add(m2row[:, :], m2row[:, :], EPS)
                nc.scalar.activation(musq[:, :], m2row[:, :], AF.Sqrt)
                rrow = tp.tile([1, CH], F32R, tag="rrow")
                nc.vector.reciprocal(rrow[:, :], musq[:, :])
                mrs = tp.tile([1, CH], F32R, tag="mrs")
                nc.vector.tensor_tensor(mrs[:, :], murow[:, :], rrow[:, :], ALU.mult)
                ps_rs = psS.tile([128, CH], F32, tag="ps")
                nc.tensor.matmul(ps_rs[:, :], onesB[:, :], rrow[:, :], start=True, stop=True)
                ps_mrs = psS.tile([128, CH], F32, tag="ps")
                nc.tensor.matmul(ps_mrs[:, :], onesB[:, :], mrs[:, :], start=True, stop=True)
                rsb = tp.tile([128, CH], F32, tag="rsb")
                nc.vector.tensor_copy(rsb[:, :], ps_rs[:, :])
                mrsb = tp.tile([128, CH], F32, tag="mrsb")
                nc.vector.tensor_copy(mrsb[:, :], ps_mrs[:, :])
                for c in range(KC):
                    xs = xT[:, c * CH:(c + 1) * CH]
                    t0 = tp.tile([128, CH], F32, tag="t0")
                    eng = nc.vector if c % 2 == 0 else nc.gpsimd
                    eng.tensor_tensor(t0[:, :], xs, rsb[:, :], ALU.mult)
                    eng.tensor_tensor(t0[:, :], t0[:, :], mrsb[:, :], ALU.subtract)
                    nc.scalar.activation(
                        hT[:, c * CH:(c + 1) * CH], t0[:, :],
                        AF.Square, scale=float(ADALN_K ** 0.5), bias=kb_t[:, 0:1])

            def emit_ag():
                ag_in = dr.tile([TG, CH], F16, tag="agi")
                nc.sync.dma_start(
                    ag_in[:, :].rearrange("(c q) t -> q c t", q=128),
                    hT[:, :].rearrange("q (c t) -> q c t", c=KC))
                ag_out = dr.tile([4 * TG, CH], F16, tag="ago")
                if SKIP_AG:
                    nc.sync.dma_start(ag_out[0:TG, :], ag_in[:, :])
                else:
                    nc.gpsimd.collective_compute(
                        "AllGather", ALU.bypass, replica_groups=RG,
                        ins=[ag_in.opt()], outs=[ag_out.opt()])
                return ag_out

            def load_qkv_weights(layer):
                wqkT = wqp.tile([128, 4 * KC * 128], F16, tag="wqk")
                nc.sync.dma_start(
                    wqkT[:, :].rearrange("p (mb kc c) -> p mb kc c", mb=4, kc=KC),
                    wqk_d[layer].rearrange("mb p kc c -> p mb kc c"))
                wvL = wvp.tile([128, KC * 256], F16, tag="wv")
                nc.sync.dma_start(
                    wvL[:, :].rearrange("p (kc c) -> p kc c", kc=KC), wv_d[layer])
                woutT = wop.tile([128, 2 * 8 * 128], F16, tag="wout")
                nc.sync.dma_start(
                    woutT[:, :].rearrange("p (kk mb c) -> p kk mb c", kk=2, mb=8),
                    wout_d[layer].rearrange("kk p mb c -> p kk mb c"))
                return wqkT, wvL, woutT

            def load_w1_chunk(layer, mc):
                w1c = wf1.tile([128, 8 * KC * 128], F16, tag="wf1")
                nc.sync.dma_start(
                    w1c[:, :].rearrange("p (mb kc c) -> p mb kc c", mb=8, kc=KC),
                    w1_d[layer, mc * 8:(mc + 1) * 8].rearrange("mb p kc c -> p mb kc c"))
                return w1c

            def load_w2_strip(layer, mb):
                wt = wf2.tile([128, 32 * 128], F16, tag="wf2")
                nc.sync.dma_start(
                    wt[:, :].rearrange("p (k c) -> p k c", k=32), w2_d[layer, mb])
                return wt

            def emit_unstage(ag_out):
                # unstage: hTg[q, c*TG + p*CH + t] = ag_out[p*TG + c*128 + q, t]
                for p in range(4):
                    nc.sync.dma_start(
                        hTg[:, :].rearrange("q (c p t) -> q c p t", c=KC, p=4)[:, :, p],
                        ag_out[p * TG:(p + 1) * TG, :].rearrange("(c q) t -> q c t", q=128))

            def emit_qkv(layer, wqkT, wvL):
                qkb = consts["qkb"]
                # piece-major so attention on early blocks can start sooner
                for p in range(4):
                    for dst, mb, m in [(kT2, 2, 0), (qT2, 0, 0), (kT2, 3, 1), (qT2, 1, 1)]:
                        pq = psS.tile([128, 256], F32, tag="ps")
                        for kc in range(KC):
                            nc.tensor.matmul(
                                pq[:, :],
                                wqkT[:, (mb * KC + kc) * 128:(mb * KC + kc + 1) * 128],
                                hTg[:, kc * TG + p * 256: kc * TG + (p + 1) * 256],
                                start=(kc == 0), stop=(kc == KC - 1))
                        nc.scalar.activation(
                            dst[:, m * TG + p * 256: m * TG + (p + 1) * 256],
                            pq[:, :], AF.Identity, bias=qkb[:, mb:mb + 1])
                    for jb in (2 * p, 2 * p + 1):
                        pv = psS.tile([128, 256], F32, tag="ps")
                        for kc in range(KC):
                            nc.tensor.matmul(
                                pv[:, :],
                                hTg[:, kc * TG + jb * 128: kc * TG + jb * 128 + 128],
                                wvL[:, kc * 256:(kc + 1) * 256],
                                start=(kc == 0), stop=(kc == KC - 1))
                        nc.vector.tensor_scalar_mul(
                            vstg[:, jb * 260:(jb + 1) * 260]
                            .rearrange("p (h x) -> p h x", x=65)[:, :, 0:64],
                            pv[:, :].rearrange("p (h d) -> p h d", d=64),
                            mqp8_t[:, jb:jb + 1])

            def emit_attn(layer):
                """Paired i-blocks: unit (m, hp, a, cj) covers i in {2a, 2a+1},
                j-blocks {2cj, 2cj+1}; all e/av/normalize ops 256 wide."""
                units = []
                for m in range(2):
                    for hp in range(2):
                        for a in range(4):
                            for cj in range(a + 1):
                                units.append((m, hp, a, cj, a + 1))
                po_state = {}

                def stage_A(u):
                    m, hp, a, cj, njc = u
                    prow = hp * 64
                    icol = m * TG + 2 * a * 128
                    pe = pmid.tile([128, 512], F32, tag="pmid")
                    for t in range(2):
                        jb = 2 * cj + t
                        sl = pe[:, t * 256:(t + 1) * 256]
                        is_d = jb >= 2 * a          # j-block hits the i-pair diag
                        nc.tensor.matmul(
                            sl,
                            kT2[prow:prow + 64, m * TG + jb * 128: m * TG + jb * 128 + 128],
                            qT2[prow:prow + 64, icol: icol + 256],
                            start=True, stop=not is_d, skip_group_check=True)
                        if is_d:
                            nc.tensor.matmul(
                                sl, ident_t[:, :],
                                dmask2_t[:, (jb - 2 * a) * 256:(jb - 2 * a + 1) * 256],
                                start=False, stop=True, skip_group_check=True)
                    aT = tp4.tile([128, 512], BF16, tag="aT")
                    nc.scalar.activation(aT[:, :], pe[:, :], AF.Exp, scale=SCALE)
                    return aT

                def stage_B(u, aT):
                    m, hp, a, cj, njc = u
                    h = 2 * m + hp
                    if cj == 0:
                        po = pso.tile([65, 256], F32, tag="po")
                        po_state[(m, hp, a)] = po
                        # denominator floor: po[64,:] += 1e-30 (tiny65 col 64)
                        nc.tensor.matmul(po[:, :], tiny65[:, :], ones256[:, :],
                                         start=True, stop=False, skip_group_check=True)
                    po = po_state[(m, hp, a)]
                    for t in range(2):
                        jb = 2 * cj + t
                        nc.tensor.matmul(
                            po[:, :],
                            vstg[:, jb * 260 + h * 65: jb * 260 + h * 65 + 65],
                            aT[:, t * 256:(t + 1) * 256],
                            start=False, stop=(jb == 2 * a + 1), skip_group_check=True)
                    if cj == njc - 1:
                        stage_C(u, po)

                def stage_C(u, po):
                    m, hp, a, cj, njc = u
                    prow = hp * 64
                    drow = tp.tile([1, 256], F32R, tag="drow2")
                    nc.vector.reciprocal(drow[:, :], po[64:65, :])
                    pb = pso.tile([64, 256], F32, tag="po")
                    nc.tensor.matmul(pb[:, :], onesB[0:1, 0:64], drow[:, :],
                                     start=True, stop=True)
                    rb = tp.tile([64, 256], F32, tag="rb")
                    nc.vector.tensor_copy(rb[:, :], pb[:, :])
                    nc.vector.tensor_tensor(
                        oT[prow:prow + 64, m * TG + 2 * a * 128: m * TG + 2 * a * 128 + 256],
                        po[0:64, :], rb[:, :], ALU.mult)

                # software pipeline, lookahead 2
                pend = []
                for u in units:
                    aT = stage_A(u)
                    pend.append((u, aT))
                    if len(pend) > 2:
                        pu, paT = pend.pop(0)
                        stage_B(pu, paT)
                for pu, paT in pend:
                    stage_B(pu, paT)

            def emit_outproj(layer, woutT):
                """Out-proj partials (+bout/4 folded in), stage, ReduceScatter."""
                bst = consts[0]
                rs_in = dr.tile([4 * TG, CH], F16, tag="rsi")
                for mb in range(8):
                    for half in range(2):
                        pq = pmid.tile([128, 512], F32, tag="pmid")
                        for kk in range(2):
                            nc.tensor.matmul(
                                pq[:, :],
                                woutT[:, (kk * 8 + mb) * 128:(kk * 8 + mb + 1) * 128],
                                oT[:, kk * TG + half * 512: kk * TG + (half + 1) * 512],
                                start=(kk == 0), stop=(kk == 1))
                        if mb % 2 == 0:
                            nc.scalar.activation(
                                outP[:, mb * TG + half * 512: mb * TG + (half + 1) * 512],
                                pq[:, :], AF.Identity, bias=bst[:, mb:mb + 1])
                        else:
                            nc.vector.tensor_scalar_add(
                                outP[:, mb * TG + half * 512: mb * TG + (half + 1) * 512],
                                pq[:, :], bst[:, mb:mb + 1])
                    nc.sync.dma_start(
                        rs_in[:, :].rearrange("(p mb q) t -> q mb p t", p=4, mb=8)[:, mb],
                        outP[:, mb * TG:(mb + 1) * TG].rearrange("q (p t) -> q p t", p=4))
                rs_out = dr.tile([TG, CH], F16, tag="rso")
                if SKIP_AG:
                    nc.sync.dma_start(rs_out[:, :], rs_in[0:TG, :])
                else:
                    nc.gpsimd.collective_compute(
                        "ReduceScatter", ALU.add, replica_groups=RG,
                        ins=[rs_in.opt()], outs=[rs_out.opt()])
                return rs_out

            def emit_res_attn(rs_out):
                nc.sync.dma_start(
                    resT[:, :].rearrange("q (c t) -> q c t", c=KC),
                    rs_out[:, :].rearrange("(c q) t -> q c t", q=128))
                for c in range(KC):
                    xs = xT[:, c * CH:(c + 1) * CH]
                    nc.vector.tensor_tensor(xs, xs, resT[:, c * CH:(c + 1) * CH], ALU.add)
                    nc.gpsimd.tensor_tensor(xs, xs, mbT[:, :], ALU.mult)

            def emit_ffn(layer, w1cs, w2s):
                if SKIP_FFN:
                    return
                b1t = consts["b1"]
                for mc in range(4):
                    if mc not in w1cs:
                        w1cs[mc] = load_w1_chunk(layer, mc)
                    w1c = w1cs[mc]
                    for mbl in range(8):
                        mb = mc * 8 + mbl
                        pf = psS.tile([128, CH], F32, tag="ps")
                        for kc in range(KC):
                            nc.tensor.matmul(
                                pf[:, :],
                                w1c[:, (mbl * KC + kc) * 128:(mbl * KC + kc + 1) * 128],
                                hT[:, kc * CH:(kc + 1) * CH],
                                start=(kc == 0), stop=(kc == KC - 1))
                        nc.scalar.activation(ffT[:, mb * CH:(mb + 1) * CH], pf[:, :],
                                             AF.Gelu, bias=b1t[:, mb:mb + 1])
                bst = consts[1]
                for mb in range(8):
                    wt = w2s[mb] if mb in w2s else load_w2_strip(layer, mb)
                    pq = psS.tile([128, CH], F32, tag="ps")
                    for kk in range(32):
                        nc.tensor.matmul(pq[:, :], wt[:, kk * 128:(kk + 1) * 128],
                                         ffT[:, kk * CH:(kk + 1) * CH],
                                         start=(kk == 0), stop=(kk == 31))
                    xs = xT[:, mb * CH:(mb + 1) * CH]
                    nc.vector.scalar_tensor_tensor(xs, pq[:, :], bst[:, mb:mb + 1], xs,
                                                   ALU.add, ALU.add)
                    nc.gpsimd.tensor_tensor(xs, xs, mbT[:, :], ALU.mult)

            # ---- main loop ----
            for rep in range(REPS):
                for layer in range(L):
                    load_layer_consts(layer)
                    emit_adaln(layer, 0)
                    ag_out = emit_ag()
                    # weight prefetch dispatches on SP while the AG runs
                    wqkT, wvL, woutT = load_qkv_weights(layer)
                    w1cs = {0: load_w1_chunk(layer, 0), 1: load_w1_chunk(layer, 1)}
                    w2s = {0: load_w2_strip(layer, 0), 1: load_w2_strip(layer, 1)}
                    emit_unstage(ag_out)
                    emit_qkv(layer, wqkT, wvL)
                    emit_attn(layer)
                    rs_out = emit_outproj(layer, woutT)
                    emit_res_attn(rs_out)
                    emit_adaln(layer, 1)
                    emit_ffn(layer, w1cs, w2s)

            nc.sync.dma_start(
                out_d[:, :].bitcast(F32R).rearrange("(c q) t -> q c t", q=128),
                xT[:, :].rearrange("q (c t) -> q c t", c=KC))

    nc.finalize()
    return nc


def get_nc():
    if "nc" not in _CACHED:
        _CACHED["nc"] = _build_nc()
    return _CACHED["nc"]


def _rearr(v, nch):
    """(..., nch*128) -> (..., 128, nch)."""
    v = np.asarray(v, dtype=np.float32)
    return np.ascontiguousarray(v.reshape(*v.shape[:-1], nch, 128).swapaxes(-1, -2))


def _strips(w, nmb, nkc):
    """[L, K, M] -> [L, nmb, 128, nkc, 128] fp16, [l,mb,p,kc,c]=w[l,kc*128+p,mb*128+c]."""
    Lw = w.shape[0]
    a = w.reshape(Lw, nkc, 128, nmb, 128).transpose(0, 3, 2, 1, 4)
    return np.ascontiguousarray(a.astype(np.float16))


def make_in_maps(x, m, l, Wqkv, Wout, bout, adaln_attn, adaln_ffn, W1, b1, W2, b2):
    x = np.asarray(x, np.float32)
    m = np.asarray(m, np.float32)
    l = np.asarray(l)
    Wqkv = np.asarray(Wqkv, np.float32)
    Wout = np.asarray(Wout, np.float32)
    bout = np.asarray(bout, np.float32)
    adaln_attn = np.asarray(adaln_attn, np.float32)
    adaln_ffn = np.asarray(adaln_ffn, np.float32)
    W1 = np.asarray(W1, np.float32)
    b1 = np.asarray(b1, np.float32)
    W2 = np.asarray(W2, np.float32)
    b2 = np.asarray(b2, np.float32)

    onescol = np.ones((128, 1), np.float32)
    kbias = np.full((128, 1), -1.0 / (2.0 * ADALN_K ** 0.5), np.float32)
    pp, cc = np.meshgrid(np.arange(128), np.arange(128), indexing="ij")
    tri = np.where(pp > cc, -30000.0, 0.0).astype(np.float16)
    zz = np.zeros((128, 128), np.float16)
    kill = np.full((128, 128), -30000.0, np.float16)
    # [jb==2a: (tri | 0)] [jb==2a+1: (kill | tri)] for the i-pair columns
    diagm = np.concatenate([tri, zz, kill, tri], axis=1)   # [128, 512]
    identm = np.eye(128, dtype=np.float16)
    tiny65 = np.zeros((128, 65), np.float32)
    tiny65[:, 64] = 1e-30 / 128.0

    per_batch = {}
    for b in range(2):
        lv = int(l[b])
        ga = adaln_attn[:, lv, :]
        gf = adaln_ffn[:, lv, :]
        g1a = (2.0 * np.exp(ga[:, :D])).astype(np.float32)
        g1f = (2.0 * np.exp(gf[:, :D])).astype(np.float32)
        bea = (ga[:, D:] + g1a / (4.0 * ADALN_K)).astype(np.float32)
        bef = (gf[:, D:] + g1f / (4.0 * ADALN_K)).astype(np.float32)
        g1a, g1f = -g1a, -g1f
        wqkv_s = Wqkv * g1a[:, :, None]
        w1_s = W1 * g1f[:, :, None]
        wv_full = Wqkv[:, :, 2 * D:3 * D]
        vc = np.einsum("ldf,ld->lf", wv_full, bea)
        bout_c = bout + np.einsum("ldf,ld->lf", Wout, vc)
        qkbias = np.einsum("ldf,ld->lf", Wqkv[:, :, :2 * D], bea).astype(np.float32)
        b1_c = (b1 + np.einsum("ldf,ld->lf", W1, bef)).astype(np.float32)
        w1_r = _strips(w1_s, 32, KC)
        w2_r = _strips(W2, 8, 32)
        # bout added by all 4 cores pre-ReduceScatter -> divide by group size
        bsum_t = _rearr(np.stack([bout_c * 0.25, b2], axis=1), 8)
        per_batch[b] = dict(wqkv_s=wqkv_s, qkbias=qkbias, w1=w1_r, w2=w2_r,
                            b1t=_rearr(b1_c, 32), bsum=bsum_t)

    in_maps = []
    for core in range(8):
        b, r = core // 4, core % 4
        pb = per_batch[b]
        hb = 4 * r * 64                    # first q/k/v column of my heads
        wqkv_s = pb["wqkv_s"]
        wq = _strips(wqkv_s[:, :, hb:hb + 256], 2, KC)
        wk = _strips(wqkv_s[:, :, D + hb:D + hb + 256], 2, KC)
        wqk_r = np.concatenate([wq, wk], axis=1)           # [L, 4, 128, KC, 128]
        wv_sl = wqkv_s[:, :, 2 * D + hb:2 * D + hb + 256]  # [L, D, 256]
        wv_r = np.ascontiguousarray(
            wv_sl.reshape(L, KC, 128, 256).transpose(0, 2, 1, 3).astype(np.float16))
        wout_r = np.ascontiguousarray(
            Wout[:, hb:hb + 256, :].reshape(L, 2, 128, 8, 128).astype(np.float16))
        qkb4 = np.stack([pb["qkbias"][:, hb:hb + 128],
                         pb["qkbias"][:, hb + 128:hb + 256],
                         pb["qkbias"][:, D + hb:D + hb + 128],
                         pb["qkbias"][:, D + hb + 128:D + hb + 256]], axis=1)  # [L,4,128]
        qkb_r = np.ascontiguousarray(qkb4.transpose(0, 2, 1))                  # [L,128,4]
        xTc = np.ascontiguousarray(x[b].T[:, r * CH:(r + 1) * CH])
        mrow = np.ascontiguousarray(m[b, r * CH:(r + 1) * CH, 0].reshape(1, CH))
        mqp8 = np.ascontiguousarray(
            m[b, :, 0].reshape(8, 128).T.astype(np.float32))   # [128, 8]
        in_maps.append({
            "xT": xTc, "wqk": wqk_r, "wv": wv_r, "wout": wout_r,
            "w1": pb["w1"], "w2": pb["w2"], "qkb": qkb_r, "b1t": pb["b1t"],
            "bsum": pb["bsum"], "mqp8": mqp8, "mrow": mrow,
            "onescol": onescol, "kbias": kbias, "diagm": diagm,
            "identm": identm, "tiny65": tiny65,
        })
    return in_maps


def kernel(**inputs):
    nc = get_nc()
    in_maps = make_in_maps(**inputs)
    res = run_bass_kernel_spmd(nc, in_maps, core_ids=list(range(8)))
    out = np.zeros((2, T, D), np.float32)
    for core in range(8):
        b, r = core // 4, core % 4
        o = res.results[core]["out_xT"]          # [D, CH]
        out[b, r * CH:(r + 1) * CH, :] = o.T
    return np.ascontiguousarray(out)


# revision 29
# speedup vs baseline: 1.2926x; 1.2926x over previous
"""Trainium2 Bass kernel for the 4-layer adaLN causal transformer (v3).

v3: attention restructured for instruction efficiency: e-matmuls write wide
PSUM groups ([128,1024]/[128,640] per head), ONE exp per group, bf16
kill-mask multiplies (causal block-kill + diag triangle) on DVE/Pool, key
padding folded into zeroed v rows/ones at staging, softmax denominators
processed per head-pair.

Sharding: sequence-parallel. 8 cores = 2 batch groups x 4 token shards.
Core (b, s) owns query blocks {s, 7-s} of batch b (zigzag for causal load
balance; every core sees exactly 9 key-blocks of true attention work).
Weights are replicated (full, adaLN-folded per batch, fp16); there are NO
AllReduces. Per layer the only collectives are two small AllGathers (k and
v across the 4 shards), and the diagonal attention blocks use local k/v so
they start before the AllGather lands.

Activations stay transposed xT[d, t_local] (f32r residual stream, fp16
modulated hT). The adaLN affine is folded into Wqkv/W1 on the host exactly
as in the TP kernel: gamma scales weight rows, beta enters as per-partition
biases on q/k and the gelu, and the v-beta contribution folds into bout.
Attention avoids max-subtraction: exp() rides the ACT bias with -1e30 key
masks; aT/v are bf16 (range) while everything else is fp16.

Residuals are masked every sub-block ((x+f)*m, matching the reference), so
padded-token columns stay exactly 0 and nothing can overflow -> no clamps.

Uniform SPMD program: per-core differences live only in input data
(madd2/mdiag mask tables, xT slices); off-diagonal passes that a core does
not need are killed by -1e30 masks (~25% wasted attention work).
"""

import os
import numpy as np

import concourse.bacc as bacc
import concourse.mybir as mybir
from concourse.tile import TileContext
from concourse.bass_utils import run_bass_kernel_spmd

F32 = mybir.dt.float32
F32R = mybir.dt.float32r
F16 = mybir.dt.float16
BF16 = mybir.dt.bfloat16
FP8 = mybir.dt.float8e4
AF = mybir.ActivationFunctionType
ALU = mybir.AluOpType

D = 1024
T = 1024
L = 4
CH = 256            # local tokens per core (2 blocks of 128)
KC = 8              # d_model chunks
ADALN_K = 0.1
EPS = 1e-5
KSC = 8.0           # k/v pre-scale into fp8e4m3 range for the AllGather
                    # (must keep KSC*(k+bias) well under 448: e4m3 saturates to NaN)
SCALE = 0.125 / KSC
RG = [[0, 1, 2, 3], [4, 5, 6, 7]]
NJ0 = 3             # uniform off-diag key blocks for iq=0 (covers qb=s<=3)
NJ1 = 7             # for iq=1 (covers qb=7-s<=7)
REPS = int(os.environ.get("BK2_REPS", "1"))
SKIP_AG = bool(int(os.environ.get("BK2_SKIP_AG", "0")))    # timing ablation
SKIP_ATTN = bool(int(os.environ.get("BK2_SKIP_ATTN", "0")))
SKIP_FFN = bool(int(os.environ.get("BK2_SKIP_FFN", "0")))
WONCE = bool(int(os.environ.get("BK2_WONCE", "0")))        # timing ablation: 1 DMA per weight kind/layer
EXP_DVE = bool(int(os.environ.get("BK2_EXP_DVE", "0")))    # timing ablation: exp -> DVE copy

_CACHED = {}


def _build_nc():
    nc = bacc.Bacc(target_bir_lowering=False, debug=False)

    xT_d = nc.dram_tensor("xT", [D, CH], F32, kind="ExternalInput")
    # weights pre-rearranged on host: [L, mb, kc, p, c] so strips DMA contiguously
    wqk_d = nc.dram_tensor("wqk", [L, 16, 128, KC, 128], F16, kind="ExternalInput")
    wv_d = nc.dram_tensor("wv", [L, 128, KC, D], F16, kind="ExternalInput")
    wout_d = nc.dram_tensor("wout", [L, 8, 128, KC, 128], F16, kind="ExternalInput")
    w1_d = nc.dram_tensor("w1", [L, 32, 128, KC, 128], F16, kind="ExternalInput")
    w2_d = nc.dram_tensor("w2", [L, 8, 128, 32, 128], F16, kind="ExternalInput")
    qkb_d = nc.dram_tensor("qkb", [L, 128, 16], F32, kind="ExternalInput")
    b1t_d = nc.dram_tensor("b1t", [L, 128, 32], F32, kind="ExternalInput")
    bsum_d = nc.dram_tensor("bsum", [L, 2, 128, 8], F32, kind="ExternalInput")
    kmask_d = nc.dram_tensor("kmask", [128, 1664], BF16, kind="ExternalInput")
    mqp_d = nc.dram_tensor("mqp", [128, 2], F32, kind="ExternalInput")
    mrow_d = nc.dram_tensor("mrow", [1, CH], F32, kind="ExternalInput")
    ones_d = nc.dram_tensor("onescol", [128, 1], F32, kind="ExternalInput")
    kb_d = nc.dram_tensor("kbias", [128, 1], F32, kind="ExternalInput")
    out_d = nc.dram_tensor("out_xT", [D, CH], F32, kind="ExternalOutput")

    with TileContext(nc) as tc:
        with nc.allow_low_precision("fp16/bf16 intermediates by design"), \
             tc.tile_pool(name="pers", bufs=1) as pers, \
             tc.tile_pool(name="wp", bufs=5) as wp, \
             tc.tile_pool(name="wf2", bufs=2) as wf2, \
             tc.tile_pool(name="cst", bufs=8) as cst, \
             tc.tile_pool(name="tp", bufs=3) as tp, \
             tc.tile_pool(name="tp4", bufs=6) as tp4, \
             tc.tile_pool(name="wvp", bufs=1) as wvp, \
             tc.tile_pool(name="ps", bufs=2, space="PSUM") as ps, \
             tc.tile_pool(name="pse", bufs=2, space="PSUM") as pse, \
             tc.tile_pool(name="pso", bufs=2, space="PSUM") as pso, \
             tc.tile_pool(name="dr", bufs=2, space="DRAM") as dr:

            # ---- persistent tiles ----
            xT = pers.tile([128, KC * CH], F32R, tag="xT")
            hT = pers.tile([128, KC * CH], F16, tag="hT")
            qT = pers.tile([128, 8 * CH], F16, tag="qT")       # pair m at m*CH, iq block at +iq*128
            kst = pers.tile([128, 8 * CH], F16, tag="kst")     # local k*KSC (also diag lhsT)
            kst8 = pers.tile([128, 8 * CH], FP8, tag="kst8")   # fp8 AG payload (k)
            kT8 = pers.tile([128, 64 * 128], FP8, tag="kT8")   # gathered k (fp8)
            kT = pers.tile([128, 64 * 128], F16, tag="kT")     # (pair m, jb) at (m*8+jb)*128
            vstg = pers.tile([128, 2 * 1280], BF16, tag="vstg")  # local v*KSC+ones (65/head, pad 1280)
            vstg8 = pers.tile([128, 2 * 1280], FP8, tag="vstg8")  # fp8 AG payload (v)
            vst8 = pers.tile([128, 8 * 1280], FP8, tag="vst8")    # gathered v (fp8)
            vst = pers.tile([128, 8 * 1280], BF16, tag="vst")    # AG v: (jb, h) at jb*1280+h*65
            oT = pers.tile([128, KC * CH], F16, tag="oT")
            ffT = pers.tile([128, 32 * CH], F16, tag="ffT")
            onesK = pers.tile([128, 1], F32R, tag="onesK")
            onesB = pers.tile([1, 128], F32R, tag="onesB")
            km_b = pers.tile([128, 1664], BF16, tag="kmb")
            kb_t = pers.tile([128, 1], F32, tag="kb")
            mqp_t = pers.tile([128, 2], F32, tag="mqp")
            mrow_r = pers.tile([1, CH], F32R, tag="mrow")
            mbT = pers.tile([128, CH], F32, tag="mbT")         # mask broadcast

            nc.sync.dma_start(onesK[:, :], ones_d[:, :].bitcast(F32R))
            nc.sync.dma_start(onesB[:, :], ones_d[:, 0:1].bitcast(F32R).rearrange("p 1 -> 1 p"))
            nc.sync.dma_start(km_b[:, :], kmask_d[:, :])
            nc.sync.dma_start(kb_t[:, :], kb_d[:, :])
            nc.sync.dma_start(mqp_t[:, :], mqp_d[:, :])
            nc.sync.dma_start(mrow_r[:, :], mrow_d[:, :].bitcast(F32R))
            # ones columns for the softmax denominators (x=64 of each 65-strip)
            for blk in range(2):
                ones_ap = (vstg[:, blk * 1280: blk * 1280 + 1040]
                           .rearrange("p (s x) -> p s x", x=65)[:, :, 64:65])
                nc.vector.memset(ones_ap, 1.0)
                nc.vector.tensor_scalar_mul(ones_ap, ones_ap, mqp_t[:, blk:blk + 1])
                nc.vector.memset(vstg[:, blk * 1280 + 1040:(blk + 1) * 1280], 0.0)
            for c in range(KC):
                nc.sync.dma_start(
                    xT[:, c * CH:(c + 1) * CH],
                    xT_d[c * 128:(c + 1) * 128, :].bitcast(F32R))
            # mask broadcast [128, CH]
            pm = ps.tile([128, CH], F32, tag="ps")
            nc.tensor.matmul(pm[:, :], onesB[:, :], mrow_r[:, :], start=True, stop=True)
            nc.vector.tensor_copy(mbT[:, :], pm[:, :])

            consts = {}

            def load_layer_consts(layer):
                qkb = cst.tile([128, 16], F32, tag="qkb")
                nc.sync.dma_start(qkb[:, :], qkb_d[layer])
                b1t = cst.tile([128, 32], F32, tag="b1")
                nc.sync.dma_start(b1t[:, :], b1t_d[layer])
                bs0 = cst.tile([128, 8], F32, tag="bs0")
                nc.sync.dma_start(bs0[:, :], bsum_d[layer, 0])
                bs1 = cst.tile([128, 8], F32, tag="bs1")
                nc.sync.dma_start(bs1[:, :], bsum_d[layer, 1])
                consts.update({"qkb": qkb, "b1": b1t, 0: bs0, 1: bs1})

            wcache = {}

            def wload(w_d, layer, mb, kcnt, pool, tag):
                key = id(w_d)
                if WONCE and key in wcache:
                    return wcache[key]
                wt = pool.tile([128, kcnt * 128], F16, tag=tag)
                nc.sync.dma_start(
                    wt[:, :].rearrange("p (k c) -> p k c", k=kcnt), w_d[layer, mb])
                wcache[key] = wt
                return wt

            def emit_adaln(layer, sb):
                """hT = mod(norm(xT)); affine folded into consuming weights."""
                ps_sum = ps.tile([1, CH], F32, tag="ps")
                ps_sq = ps.tile([1, CH], F32, tag="ps")
                for c in range(KC):
                    xs = xT[:, c * CH:(c + 1) * CH]
                    xsq = tp.tile([128, CH], F32R, tag="xsq")
                    nc.scalar.activation(xsq[:, :], xs, AF.Square)
                    nc.tensor.matmul(ps_sum[:, :], onesK[:, :], xs,
                                     start=(c == 0), stop=(c == KC - 1))
                    nc.tensor.matmul(ps_sq[:, :], onesK[:, :], xsq[:, :],
                                     start=(c == 0), stop=(c == KC - 1))
                murow = tp.tile([1, CH], F32R, tag="murow")
                nc.scalar.mul(murow[:, :], ps_sum[0:1, :], 1.0 / D)
                m2row = tp.tile([1, CH], F32, tag="m2row")
                nc.scalar.mul(m2row[:, :], ps_sq[0:1, :], 1.0 / D)
                musq = tp.tile([1, CH], F32, tag="musq")
                nc.vector.tensor_tensor(musq[:, :], murow[:, :], murow[:, :], ALU.mult)
                nc.vector.tensor_tensor(m2row[:, :], m2row[:, :], musq[:, :], ALU.subtract)
                nc.vector.tensor_scalar_add(m2row[:, :], m2row[:, :], EPS)
                nc.scalar.activation(musq[:, :], m2row[:, :], AF.Sqrt)
                rrow = tp.tile([1, CH], F32R, tag="rrow")
                nc.vector.reciprocal(rrow[:, :], musq[:, :])
                mrs = tp.tile([1, CH], F32R, tag="mrs")
                nc.vector.tensor_tensor(mrs[:, :], murow[:, :], rrow[:, :], ALU.mult)
                ps_rs = ps.tile([128, CH], F32, tag="ps")
                nc.tensor.matmul(ps_rs[:, :], onesB[:, :], rrow[:, :], start=True, stop=True)
                ps_mrs = ps.tile([128, CH], F32, tag="ps")
                nc.tensor.matmul(ps_mrs[:, :], onesB[:, :], mrs[:, :], start=True, stop=True)
                rsb = tp.tile([128, CH], F32, tag="rsb")
                nc.vector.tensor_copy(rsb[:, :], ps_rs[:, :])
                mrsb = tp.tile([128, CH], F32, tag="mrsb")
                nc.vector.tensor_copy(mrsb[:, :], ps_mrs[:, :])
                for c in range(KC):
                    xs = xT[:, c * CH:(c + 1) * CH]
                    t0 = tp.tile([128, CH], F32, tag="t0")
                    eng = nc.vector if c % 2 == 0 else nc.gpsimd
                    eng.tensor_tensor(t0[:, :], xs, rsb[:, :], ALU.mult)
                    eng.tensor_tensor(t0[:, :], t0[:, :], mrsb[:, :], ALU.subtract)
                    nc.scalar.activation(
                        hT[:, c * CH:(c + 1) * CH], t0[:, :],
                        AF.Square, scale=float(ADALN_K ** 0.5), bias=kb_t[:, 0:1])

            def emit_k(layer, ag_in):
                qkb = consts["qkb"]
                for m in range(8):
                    mb = 8 + m
                    wt = wload(wqk_d, layer, mb, KC, wp, "wa")
                    pq = ps.tile([128, CH], F32, tag="ps")
                    for kk in range(KC):
                        nc.tensor.matmul(pq[:, :], wt[:, kk * 128:(kk + 1) * 128],
                                         hT[:, kk * CH:(kk + 1) * CH],
                                         start=(kk == 0), stop=(kk == KC - 1))
                    # kst = KSC*(k + bias): scale into fp8 range (bias pre-scaled on host)
                    nc.scalar.activation(kst[:, m * CH:(m + 1) * CH], pq[:, :],
                                         AF.Identity, scale=KSC, bias=qkb[:, mb:mb + 1])
                    eng = nc.vector if m % 2 == 0 else nc.gpsimd
                    eng.tensor_copy(kst8[:, m * CH:(m + 1) * CH],
                                    kst[:, m * CH:(m + 1) * CH])
                    nc.sync.dma_start(ag_in[m * 128:(m + 1) * 128, :],
                                      kst8[:, m * CH:(m + 1) * CH])

            def emit_v(layer, ag_in):
                wvL = wvp.tile([128, KC * D], F16, tag="wv")
                nc.sync.dma_start(
                    wvL[:, :].rearrange("p (k c) -> p k c", k=KC), wv_d[layer])
                for blk in range(2):
                    for half in range(2):
                        pv = ps.tile([128, 512], F32, tag="ps")
                        for kc in range(KC):
                            nc.tensor.matmul(
                                pv[:, :],
                                hT[:, kc * CH + blk * 128: kc * CH + blk * 128 + 128],
                                wvL[:, kc * D + half * 512: kc * D + (half + 1) * 512],
                                start=(kc == 0), stop=(kc == KC - 1))
                        # v psum [128tok, 512 vd] -> vstg strips, zeroing pad-token rows
                        nc.vector.tensor_scalar_mul(
                            vstg[:, blk * 1280 + half * 8 * 65: blk * 1280 + (half * 8 + 8) * 65]
                            .rearrange("p (h x) -> p h x", x=65)[:, :, 0:64],
                            pv[:, :].rearrange("p (h d) -> p h d", d=64),
                            mqp_t[:, blk:blk + 1])
                    eng = nc.vector if blk == 0 else nc.gpsimd
                    eng.tensor_copy(vstg8[:, blk * 1280:(blk + 1) * 1280],
                                    vstg[:, blk * 1280:(blk + 1) * 1280])
                    nc.sync.dma_start(
                        ag_in[1024 + blk * 640: 1024 + (blk + 1) * 640, :]
                        .rearrange("(p x) c -> p (x c)", p=128),
                        vstg8[:, blk * 1280:(blk + 1) * 1280])

            def emit_q(layer):
                qkb = consts["qkb"]
                for m in range(8):
                    wt = wload(wqk_d, layer, m, KC, wp, "wa")
                    pq = ps.tile([128, CH], F32, tag="ps")
                    for kk in range(KC):
                        nc.tensor.matmul(pq[:, :], wt[:, kk * 128:(kk + 1) * 128],
                                         hT[:, kk * CH:(kk + 1) * CH],
                                         start=(kk == 0), stop=(kk == KC - 1))
                    nc.vector.tensor_scalar_add(qT[:, m * CH:(m + 1) * CH], pq[:, :],
                                                qkb[:, m:m + 1])

            def emit_ag(tag, src, rows_out):
                if SKIP_AG:
                    out = dr.tile([rows_out, src.shape[1]], src.dtype, tag=tag + "o")
                    nc.sync.dma_start(out[0:src.shape[0], :], src[:, :])
                    return out
                out = dr.tile([rows_out, src.shape[1]], src.dtype, tag=tag + "o")
                nc.gpsimd.collective_compute(
                    "AllGather", ALU.bypass, replica_groups=RG,
                    ins=[src.opt()], outs=[out.opt()])
                return out

            def emit_unstage_k(ag_out):
                for r in range(4):
                    for pos in range(2):
                        gb = r if pos == 0 else 7 - r
                        sl = (slice(None), slice(None), slice(gb * 128, (gb + 1) * 128))
                        nc.sync.dma_start(
                            kT8[:, :].rearrange("p (m j) -> p m j", m=8)[sl],
                            ag_out[r * 2304:r * 2304 + D, pos * 128:(pos + 1) * 128]
                            .rearrange("(m p) j -> p m j", p=128))
                        eng = nc.vector if pos == 0 else nc.gpsimd
                        eng.tensor_copy(
                            kT[:, :].rearrange("p (m j) -> p m j", m=8)[sl],
                            kT8[:, :].rearrange("p (m j) -> p m j", m=8)[sl])

            def emit_unstage_v(ag_out):
                for r in range(4):
                    for pos in range(2):
                        gb = r if pos == 0 else 7 - r
                        nc.sync.dma_start(
                            vst8[:, gb * 1280:(gb + 1) * 1280],
                            ag_out[r * 2304 + 1024 + pos * 640: r * 2304 + 1024 + (pos + 1) * 640, :]
                            .rearrange("(p x) c -> p (x c)", p=128))
                        eng = nc.gpsimd if pos == 0 else nc.vector
                        eng.tensor_copy(vst[:, gb * 1280:(gb + 1) * 1280],
                                        vst8[:, gb * 1280:(gb + 1) * 1280])

            def emit_attn(layer):
                if SKIP_ATTN:
                    for c in range(KC):
                        nc.vector.tensor_copy(oT[:, c * CH:(c + 1) * CH],
                                              hT[:, c * CH:(c + 1) * CH])
                    return
                for m in range(8):
                    po = pso.tile([65, 512], F32, tag="po")
                    for h2 in range(2):
                        h = 2 * m + h2
                        prow = h2 * 64
                        qs2 = qT[prow:prow + 64, m * CH:(m + 1) * CH]
                        # group 0: jb 0..3, both iq halves -> pe cols jb*256
                        pe0 = pse.tile([128, 1024], F32, tag="pe")
                        for jb in range(4):
                            nc.tensor.matmul(
                                pe0[:, jb * 256:(jb + 1) * 256],
                                kT[prow:prow + 64, (m * 8 + jb) * 128:(m * 8 + jb + 1) * 128],
                                qs2, start=True, stop=True, skip_group_check=True)
                        aT0 = tp4.tile([128, 1024], BF16, tag="aT")
                        if EXP_DVE:
                            nc.vector.tensor_copy(aT0[:, :], pe0[:, :])
                        else:
                            nc.scalar.activation(aT0[:, :], pe0[:, :], AF.Exp, scale=SCALE)
                        eng0 = nc.gpsimd if h2 == 0 else nc.vector
                        eng0.tensor_tensor(aT0[:, :], aT0[:, :], km_b[:, 0:1024], ALU.mult)
                        # group 1: jb 4..6 iq1-only at cols u*128, diag at 384+iq*128
                        pe1 = pse.tile([128, 1024], F32, tag="pe")
                        for u in range(3):
                            jb = 4 + u
                            nc.tensor.matmul(
                                pe1[:, u * 128:(u + 1) * 128],
                                kT[prow:prow + 64, (m * 8 + jb) * 128:(m * 8 + jb + 1) * 128],
                                qT[prow:prow + 64, m * CH + 128: (m + 1) * CH],
                                start=True, stop=True, skip_group_check=True)
                        for iq in range(2):
                            nc.tensor.matmul(
                                pe1[:, 384 + iq * 128: 384 + (iq + 1) * 128],
                                kst[prow:prow + 64, m * CH + iq * 128: m * CH + iq * 128 + 128],
                                qT[prow:prow + 64, m * CH + iq * 128: m * CH + iq * 128 + 128],
                                start=True, stop=True, skip_group_check=True)
                        aT1 = tp4.tile([128, 1024], BF16, tag="aT")
                        if EXP_DVE:
                            nc.vector.tensor_copy(aT1[:, 0:640], pe1[:, 0:640])
                        else:
                            nc.scalar.activation(aT1[:, 0:640], pe1[:, 0:640], AF.Exp, scale=SCALE)
                        eng1 = nc.vector if h2 == 0 else nc.gpsimd
                        eng1.tensor_tensor(aT1[:, 0:640], aT1[:, 0:640],
                                           km_b[:, 1024:1664], ALU.mult)
                        # av accumulation into po[:, h2*256 + iq*128]
                        for iq in range(2):
                            oc = h2 * 256 + iq * 128
                            units = []
                            njb = NJ0 if iq == 0 else 4
                            for jb in range(njb):
                                units.append((vst[:, jb * 1280 + h * 65: jb * 1280 + h * 65 + 65],
                                              aT0[:, jb * 256 + iq * 128: jb * 256 + iq * 128 + 128]))
                            if iq == 1:
                                for u in range(3):
                                    jb = 4 + u
                                    units.append((vst[:, jb * 1280 + h * 65: jb * 1280 + h * 65 + 65],
                                                  aT1[:, u * 128:(u + 1) * 128]))
                            units.append((vstg[:, iq * 1280 + h * 65: iq * 1280 + h * 65 + 65],
                                          aT1[:, 384 + iq * 128: 384 + (iq + 1) * 128]))
                            for ui, (lhs, rhs) in enumerate(units):
                                nc.tensor.matmul(po[:, oc:oc + 128], lhs, rhs,
                                                 start=(ui == 0), stop=(ui == len(units) - 1),
                                                 skip_group_check=True)
                    # denominators for the whole pair
                    nc.vector.tensor_scalar_add(po[64:65, :], po[64:65, :], 1e-30)
                    drow = tp.tile([1, 512], F32R, tag="drow")
                    nc.vector.reciprocal(drow[:, :], po[64:65, :])
                    pb = ps.tile([64, 512], F32, tag="ps")
                    nc.tensor.matmul(pb[:, :], onesB[0:1, 0:64], drow[:, :],
                                     start=True, stop=True)
                    rb = tp.tile([64, 512], F32, tag="rb")
                    nc.vector.tensor_copy(rb[:, :], pb[:, :])
                    for h2 in range(2):
                        nc.vector.tensor_tensor(
                            oT[h2 * 64:(h2 + 1) * 64, m * CH:(m + 1) * CH],
                            po[0:64, h2 * 256:(h2 + 1) * 256],
                            rb[0:64, h2 * 256:(h2 + 1) * 256], ALU.mult)

            def emit_res(pq, br, c):
                """x[:, c] = (x + pq + bias) * m, engines alternating by c."""
                bst = consts[br]
                xs = xT[:, c * CH:(c + 1) * CH]
                nc.vector.scalar_tensor_tensor(xs, pq[:, :], bst[:, c:c + 1], xs,
                                                ALU.add, ALU.add)
                nc.gpsimd.tensor_tensor(xs, xs, mbT[:, :], ALU.mult)

            def emit_outproj(layer):
                for mb in range(8):
                    wt = wload(wout_d, layer, mb, KC, wp, "wa")
                    pq = ps.tile([128, CH], F32, tag="ps")
                    for kk in range(KC):
                        nc.tensor.matmul(pq[:, :], wt[:, kk * 128:(kk + 1) * 128],
                                         oT[:, kk * CH:(kk + 1) * CH],
                                         start=(kk == 0), stop=(kk == KC - 1))
                    emit_res(pq, 0, mb)

            def emit_ffn(layer):
                if SKIP_FFN:
                    return
                b1t = consts["b1"]
                for mb in range(32):
                    wt = wload(w1_d, layer, mb, KC, wp, "wa")
                    pf = ps.tile([128, CH], F32, tag="ps")
                    for kk in range(KC):
                        nc.tensor.matmul(pf[:, :], wt[:, kk * 128:(kk + 1) * 128],
                                         hT[:, kk * CH:(kk + 1) * CH],
                                         start=(kk == 0), stop=(kk == KC - 1))
                    nc.scalar.activation(ffT[:, mb * CH:(mb + 1) * CH], pf[:, :],
                                         AF.Gelu, bias=b1t[:, mb:mb + 1])
                for mb in range(8):
                    wt = wload(w2_d, layer, mb, 32, wf2, "wf2")
                    pq = ps.tile([128, CH], F32, tag="ps")
                    for kk in range(32):
                        nc.tensor.matmul(pq[:, :], wt[:, kk * 128:(kk + 1) * 128],
                                         ffT[:, kk * CH:(kk + 1) * CH],
                                         start=(kk == 0), stop=(kk == 31))
                    emit_res(pq, 1, mb)

            # ---- main loop ----
            for rep in range(REPS):
                for layer in range(L):
                    load_layer_consts(layer)
                    emit_adaln(layer, 0)
                    ag_in = dr.tile([2304, CH], FP8, tag="agi")
                    emit_k(layer, ag_in)
                    emit_v(layer, ag_in)
                    ag_out = emit_ag("ag", ag_in, 4 * 2304)
                    emit_q(layer)
                    emit_unstage_k(ag_out)
                    emit_unstage_v(ag_out)
                    emit_attn(layer)
                    emit_outproj(layer)
                    emit_adaln(layer, 1)
                    emit_ffn(layer)

            for c in range(KC):
                nc.sync.dma_start(out_d[c * 128:(c + 1) * 128, :].bitcast(F32R),
                                  xT[:, c * CH:(c + 1) * CH])

    nc.finalize()
    return nc


def get_nc():
    if "nc" not in _CACHED:
        _CACHED["nc"] = _build_nc()
    return _CACHED["nc"]


def _rearr(v, nch):
    """(..., nch*128) -> (..., 128, nch)."""
    v = np.asarray(v, dtype=np.float32)
    return np.ascontiguousarray(v.reshape(*v.shape[:-1], nch, 128).swapaxes(-1, -2))


def _strips(w, nmb, nkc):
    """[L, K, M] -> [L, nmb, nkc, 128, 128] fp16 with [l,mb,kc,p,c]=w[l,kc*128+p,mb*128+c]."""
    Lw = w.shape[0]
    a = w.reshape(Lw, nkc, 128, nmb, 128).transpose(0, 3, 2, 1, 4)
    return np.ascontiguousarray(a.astype(np.float16))


def make_in_maps(x, m, l, Wqkv, Wout, bout, adaln_attn, adaln_ffn, W1, b1, W2, b2):
    x = np.asarray(x, np.float32)
    m = np.asarray(m, np.float32)
    l = np.asarray(l)
    Wqkv = np.asarray(Wqkv, np.float32)
    Wout = np.asarray(Wout, np.float32)
    bout = np.asarray(bout, np.float32)
    adaln_attn = np.asarray(adaln_attn, np.float32)
    adaln_ffn = np.asarray(adaln_ffn, np.float32)
    W1 = np.asarray(W1, np.float32)
    b1 = np.asarray(b1, np.float32)
    W2 = np.asarray(W2, np.float32)
    b2 = np.asarray(b2, np.float32)

    causal01 = (np.arange(128)[:, None] <= np.arange(128)[None, :]).astype(np.float32)
    onescol = np.ones((128, 1), np.float32)
    kbias = np.full((128, 1), -1.0 / (2.0 * ADALN_K ** 0.5), np.float32)

    per_batch = {}
    for b in range(2):
        lv = int(l[b])
        ga = adaln_attn[:, lv, :]
        gf = adaln_ffn[:, lv, :]
        g1a = (2.0 * np.exp(ga[:, :D])).astype(np.float32)
        g1f = (2.0 * np.exp(gf[:, :D])).astype(np.float32)
        # mod = (sqrt(K)t - 1/(2 sqrt(K)))^2 = -(t - K t^2) + 1/(4K): sign into
        # gamma, constant into beta.
        bea = (ga[:, D:] + g1a / (4.0 * ADALN_K)).astype(np.float32)
        bef = (gf[:, D:] + g1f / (4.0 * ADALN_K)).astype(np.float32)
        g1a, g1f = -g1a, -g1f
        wqkv_s = Wqkv * g1a[:, :, None]
        w1_s = W1 * g1f[:, :, None]
        wv_full = Wqkv[:, :, 2 * D:3 * D]
        vc = np.einsum("ldf,ld->lf", wv_full, bea)
        bout_c = bout + np.einsum("ldf,ld->lf", Wout, vc)
        qkbias = np.einsum("ldf,ld->lf", Wqkv[:, :, :2 * D], bea).astype(np.float32)
        # k staged as KSC*(k+bias) for the fp8 AllGather; ACT applies
        # scale=KSC to the psum, so the k biases must be pre-scaled too
        qkbias = qkbias.copy()
        qkbias[:, D:] *= KSC
        b1_c = (b1 + np.einsum("ldf,ld->lf", W1, bef)).astype(np.float32)

        wqk_r = _strips(wqkv_s[:, :, :2 * D], 16, KC)
        wv_r = np.ascontiguousarray(
            wqkv_s[:, :, 2 * D:].reshape(L, KC, 128, D).transpose(0, 2, 1, 3)
            .astype(np.float16))
        wout_r = _strips(Wout, 8, KC)
        w1_r = _strips(w1_s, 32, KC)
        w2_r = _strips(W2, 8, 32)
        bsum_t = _rearr(np.stack([bout_c, b2], axis=1), 8)
        per_batch[b] = dict(
            wqk=wqk_r, wv=wv_r, wout=wout_r, w1=w1_r, w2=w2_r,
            qkb=_rearr(qkbias, 16), b1t=_rearr(b1_c, 32), bsum=bsum_t)

    in_maps = []
    for core in range(8):
        b, s = core // 4, core % 4
        blocks = [s, 7 - s]
        pb = per_batch[b]
        cols = np.concatenate([np.arange(bk * 128, (bk + 1) * 128) for bk in blocks])
        xTc = np.ascontiguousarray(x[b].T[:, cols])
        mrow = np.ascontiguousarray(m[b, cols, 0].reshape(1, CH))
        # KSC folds into v rows and the denominator ones-column (ratio exact)
        mqp = KSC * np.stack([m[b, bk * 128:(bk + 1) * 128, 0] for bk in blocks],
                             axis=1).astype(np.float32)
        kmask = np.zeros((128, 1664), np.float32)
        for jb in range(4):                      # group 0: both iq halves
            for iq, qb in enumerate(blocks):
                if jb < qb:
                    kmask[:, jb * 256 + iq * 128: jb * 256 + (iq + 1) * 128] = 1.0
        for u in range(3):                       # group 1: iq1-only jb 4..6
            if 4 + u < blocks[1]:
                kmask[:, 1024 + u * 128: 1024 + (u + 1) * 128] = 1.0
        for iq in range(2):                      # group 1: diag causal triangles
            kmask[:, 1408 + iq * 128: 1408 + (iq + 1) * 128] = causal01
        import ml_dtypes
        in_maps.append({
            "xT": xTc, "wqk": pb["wqk"], "wv": pb["wv"], "wout": pb["wout"],
            "w1": pb["w1"], "w2": pb["w2"], "qkb": pb["qkb"], "b1t": pb["b1t"],
            "bsum": pb["bsum"], "kmask": kmask.astype(ml_dtypes.bfloat16),
            "mqp": mqp, "mrow": mrow,
            "onescol": onescol, "kbias": kbias,
        })
    return in_maps


def kernel(**inputs):
    nc = get_nc()
    in_maps = make_in_maps(**inputs)
    res = run_bass_kernel_spmd(nc, in_maps, core_ids=list(range(8)))
    out = np.zeros((2, T, D), np.float32)
    for core in range(8):
        b, s = core // 4, core % 4
        o = res.results[core]["out_xT"]          # [D, CH]
        for iq, bk in enumerate([s, 7 - s]):
            out[b, bk * 128:(bk + 1) * 128, :] = o[:, iq * 128:(iq + 1) * 128].T
    return np.ascontiguousarray(out)

